# revision 22
# baseline (speedup 1.0000x reference)
"""Trainium2 Bass kernel for nn_CodecTransformerLayer (sparse window attention
+ GQA + ALiBi + SwiGLU FFN), 8-core data-parallel with forward-halo recompute.

Sharding: batch(2) x seq-block(4) = 8 shards, one per core. Each core computes
its own 512 tokens end-to-end; attention needs K/V for the next 512 tokens
(window is forward-looking: dist = j - i in [0, 512]), which the core
recomputes from a 512-token halo of x instead of communicating.

v2 changes vs the bf16 baseline (580us):
 - All big GEMMs (q/k/v, wo, w1/w3/w2, AV) run fp8e4 with DoubleRow perf
   mode (2 contraction rows per PE cell). Weights are pre-scaled by powers
   of two into fp8 range; the descales fold into existing affine ops
   (q/k LN is scale-invariant; attn_scale/ffn_scale absorb the rest), so
   the fp8 path costs zero extra instructions. The residual path stays
   exact fp32; the attention/FFN contributions are scaled by 1e-5 so fp8
   noise is invisible at the output.
 - FFN weights stream as fully-contiguous 2-4KB/partition DMAs (the
   baseline's interleaved 256B runs ran at ~1/8 DMA bandwidth and were the
   top stall).
 - Band masking is two DVE adds of precomputed 0/-1e9 tiles into the score
   PSUM before exp (the baseline's gpsimd affine_selects were ~6us/unit).
 - LayerNorm rstd = exp(-0.5*ln(var+eps)) keeps every activation in the
   natural_log_exp table set (no sqrt-set switches, no slow DVE
   reciprocal); softmax uses reciprocal_approx_fast on the [1,256]
   denominator row before PE-broadcast.
 - x is read as f32r directly (bit-identical to f32) so LN stats matmuls
   need no staging copies. SwiGLU's silu runs as tanh (same table set
   group) + two fused scalar_tensor_tensor DVE ops.
"""

import math

import numpy as np
import ml_dtypes

import concourse.bass as bass
import concourse.mybir as mybir
import concourse.tile as tile
from concourse import bacc
from concourse.bass_utils import run_bass_kernel_spmd

P = 128
DIM = 1024
N_HEADS = 16
N_KV = 8
HD = 64
HIDDEN = 4096
WINDOW = 512
NORM_EPS = 1e-5
QK_EPS = 1e-6
B = 2
S = 2048
T_OWN = 512          # tokens owned per core
T_HALO = 1024        # own + forward halo
DS = DIM // P        # 8 d-subtiles
KS = DIM // P        # 8 hd-subtiles for wo contraction
HS = HIDDEN // P     # 32 hidden subtiles
VG = HD + 2          # per-kv-head stride inside vext (64 v + ones + pad)

F32 = mybir.dt.float32
F32R = mybir.dt.float32r
BF16 = mybir.dt.bfloat16
FP8 = mybir.dt.float8e4
AF = mybir.ActivationFunctionType
OP = mybir.AluOpType
DR = mybir.MatmulPerfMode.DoubleRow

# power-of-two fp8 range scaling (descale locations in comments)
SQK = 64.0           # wq/wk premul; removed for free by the q/k layernorm
SV = 32.0            # wv premul; descale folded into asc
SWO = 32.0           # wo premul; descale folded into asc
SW1 = 64.0           # w1 premul; descale inside the tanh activation scale
SW3 = 64.0           # w3 premul; descale inside the g fuse constant
SG = 16.0            # gT fp8 premul; descale folded into fsc
SW2 = 64.0           # w2 premul; descale folded into fsc
G_FUSE = SG / (2.0 * SW1 * SW3)   # (th+1)*u*psw -> g scale
EXP_BIAS = -math.log(4.0)         # exp output /4 to keep fp8 < 240
MROW_SCALE = 16.0    # mean-row fp8 range boost; 1/16 folded into wqx/wkx/wvx
NEG_BIG = -1.0e9


def _alibi_slopes(n):
    start = 2.0 ** (-(2.0 ** (-(math.log2(n) - 3))))
    return [start * start ** i for i in range(n)]


SLOPES = _alibi_slopes(N_HEADS)


# ---------------------------------------------------------------------------
# device kernel
# ---------------------------------------------------------------------------

def _build_nc(apply_qkw):
    nc = bacc.Bacc("TRN2")

    ins = {}
    ins["xT"] = nc.dram_tensor("xT", [P, DS, T_HALO], F32R, kind="ExternalInput")
    ins["wq"] = nc.dram_tensor("wq", [2, P, 4, DS, 128], FP8, kind="ExternalInput")
    ins["wk"] = nc.dram_tensor("wk", [1, P, 4, DS, 128], FP8, kind="ExternalInput")
    ins["wv"] = nc.dram_tensor("wv", [P, DS, 512], FP8, kind="ExternalInput")
    ins["wo"] = nc.dram_tensor("wo", [P, KS, DIM], FP8, kind="ExternalInput")
    ins["w1"] = nc.dram_tensor("w1", [HS // 8, P, 4, 2, DS, 128], FP8,
                               kind="ExternalInput")
    ins["w3"] = nc.dram_tensor("w3", [HS // 8, P, 4, 2, DS, 128], FP8,
                               kind="ExternalInput")
    ins["w2"] = nc.dram_tensor("w2", [DS // 2, P, 2, HS, 128], FP8, kind="ExternalInput")
    # qnw*knw folded, head-local layout [64(pad128), head]
    ins["qkw"] = nc.dram_tensor("qkw", [P, N_HEADS], F32, kind="ExternalInput")
    ins["asc"] = nc.dram_tensor("asc", [P, DS], F32, kind="ExternalInput")
    ins["fsc"] = nc.dram_tensor("fsc", [P, DS], F32, kind="ExternalInput")
    ins["x8"] = nc.dram_tensor("x8", [P, DS, T_HALO], FP8,
                               kind="ExternalInput")
    ins["wqx"] = nc.dram_tensor("wqx", [2, DS, 128], FP8, kind="ExternalInput")
    ins["wkx"] = nc.dram_tensor("wkx", [2, 4, 128], FP8, kind="ExternalInput")
    ins["wvx"] = nc.dram_tensor("wvx", [2, 512], FP8, kind="ExternalInput")
    ins["kal"] = nc.dram_tensor("kal", [2, T_HALO], F32R, kind="ExternalInput")
    ins["qal"] = nc.dram_tensor("qal", [2, 2, N_HEADS, 256], F32R,
                                kind="ExternalInput")

    out = nc.dram_tensor("out", [P, DS, T_OWN], F32, kind="ExternalOutput")

    with tile.TileContext(nc) as tc:
        _emit(nc, tc, ins, out, apply_qkw)
    nc.finalize()
    return nc


def _ln_coeffs(nc, pool, psm, psq, inv_n, eps_ap):
    """From sum/sumsq psums (replicated across partitions), produce
    s = rstd and b = mean * rstd, both [128, 512] f32 replicated.
    rstd = exp(-0.5 * ln(var + eps)) keeps ACT in the ln/exp table set."""
    m_ = pool.tile([P, 512], F32, tag="ln_m")
    nc.vector.tensor_scalar_mul(m_[:], psm[:], inv_n)       # mean
    t_ = pool.tile([P, 512], F32, tag="ln_t")
    nc.vector.tensor_tensor(t_[:], m_[:], m_[:], OP.mult)   # mean^2
    d_ = pool.tile([P, 512], F32, tag="ln_d")
    nc.vector.scalar_tensor_tensor(d_[:], psq[:], inv_n, t_[:],
                                   OP.mult, OP.subtract)    # var
    s_ = pool.tile([P, 512], F32, tag="ln_s")
    nc.scalar.activation(s_[:], d_[:], AF.Ln, bias=eps_ap)
    nc.scalar.activation(s_[:], s_[:], AF.Exp, scale=-0.5)
    b_ = pool.tile([P, 512], F32, tag="ln_b")
    nc.vector.tensor_tensor(b_[:], m_[:], s_[:], OP.mult)   # mean * rstd
    return s_, b_, m_


def _emit(nc, tc, ins, out, apply_qkw):
    frees = []  # keep single-tile pool handles alive; release LIFO at end

    def tile_single(shape, dtype, name):
        t, f = tc.tile(shape, dtype, name=name)
        frees.append(f)
        return t

    xT, wq, wk, wv, wo = ins["xT"], ins["wq"], ins["wk"], ins["wv"], ins["wo"]
    x8d, wqx, wkx, wvx = ins["x8"], ins["wqx"], ins["wkx"], ins["wvx"]
    w1, w3, w2 = ins["w1"], ins["w3"], ins["w2"]
    qkw, asc, fsc = ins["qkw"], ins["asc"], ins["fsc"]
    kal, qal = ins["kal"], ins["qal"]

    # --- constants (kept for the whole kernel) -----------------------------
    ones_f = tile_single([P, P], F32, name="ones_f")
    nc.vector.memset(ones_f[:], 1.0)
    ones128 = tile_single([P, P], F32R, name="ones128")
    nc.vector.tensor_copy(ones128[:], ones_f[:])
    ones_bf = tile_single([P, P], BF16, name="ones_bf")
    nc.vector.tensor_copy(ones_bf[:], ones_f[:])
    qkw_sb = tile_single([P, N_HEADS], F32, name="qkw_sb")
    nc.sync.dma_start(qkw_sb[:], qkw[:])
    asc_sb = tile_single([P, DS], F32, name="asc_sb")
    nc.sync.dma_start(asc_sb[:], asc[:])
    fsc_sb = tile_single([P, DS], F32, name="fsc_sb")
    nc.sync.dma_start(fsc_sb[:], fsc[:])
    eps_n = tile_single([P, 1], F32, name="eps_n")
    nc.vector.memset(eps_n[:], NORM_EPS)
    eps_qk = tile_single([P, 1], F32, name="eps_qk")
    nc.vector.memset(eps_qk[:], QK_EPS)
    exp_b = tile_single([P, 1], F32, name="exp_b")
    nc.vector.memset(exp_b[:], EXP_BIAS)
    identF = tile_single([P, P], F32, name="identF")
    nc.gpsimd.memset(identF[:], 0.0)
    nc.gpsimd.affine_select(identF[:], ones_f[:], pattern=[[-1, P]], base=0,
                            channel_multiplier=1, compare_op=OP.is_equal,
                            fill=0.0)
    ident8 = tile_single([P, P], BF16, name="ident8")
    nc.vector.tensor_copy(ident8[:], identF[:])
    rstdT = tile_single([P, DS], F32, name="rstdT")
    mrow = tile_single([2, T_HALO], FP8, name="mrow")
    nc.gpsimd.memset(mrow[:], 0.0)

    # band-mask tiles: 0 in-band, -1e9 out of band. mlo covers key chunks
    # kc=0,1 (enforce dist >= 0), mhi covers kc=4,5 (enforce dist <= 512);
    # dist = kc*128 + p - c for q column c in the 256-block.
    mlo2 = tile_single([P, 2, 512], F32, name="mlo2")
    nc.gpsimd.memset(mlo2[:], 0.0)
    mhi2 = tile_single([P, 2, 512], F32, name="mhi2")
    nc.gpsimd.memset(mhi2[:], 0.0)
    for kc in (0, 1):
        for hh in range(2):
            nc.gpsimd.affine_select(
                mlo2[:, kc, hh * 256:(hh + 1) * 256],
                mlo2[:, kc, hh * 256:(hh + 1) * 256],
                pattern=[[-1, 256]], base=kc * 128, channel_multiplier=1,
                compare_op=OP.is_ge, fill=NEG_BIG)
    for kc in (4, 5):
        for hh in range(2):
            nc.gpsimd.affine_select(
                mhi2[:, kc - 4, hh * 256:(hh + 1) * 256],
                mhi2[:, kc - 4, hh * 256:(hh + 1) * 256],
                pattern=[[1, 256]], base=WINDOW - kc * 128,
                channel_multiplier=-1, compare_op=OP.is_ge, fill=NEG_BIG)

    xTo = tile_single([P, DS, T_OWN], F32R, name="xTo")
    nc.sync.dma_start(xTo[:], xT[:, :, 0:T_OWN])
    aoT = tile_single([P, KS, T_OWN], FP8, name="aoT")
    x2T = tile_single([P, DS, T_OWN], F32R, name="x2T")

    woc = tile_single([P, KS, DIM], FP8, name="woc")
    nc.gpsimd.dma_start(woc[:], wo[:])

    NQ = 256

    x8, free_x8 = tc.tile([P, DS, T_HALO], FP8, name="x8")
    nc.scalar.dma_start(x8[:], x8d[:])

    # ======================================================================
    # Phase 1: attn LN over halo tokens -> hT (fp8)
    # (attn_norm_w is folded into wq/wk/wv on the host)
    # ======================================================================
    xTh, free_xTh = tc.tile([P, DS, T_OWN], F32R, name="xTh")
    nc.scalar.dma_start(xTh[:], xT[:, :, T_OWN:T_HALO])
    with tc.tile_pool(name="p1c", bufs=3) as p1c, \
         tc.tile_pool(name="p1s", bufs=2) as p1s, \
         tc.tile_pool(name="psA1", bufs=2, space="PSUM") as psA1, \
         tc.tile_pool(name="psT1", bufs=2, space="PSUM") as psT1:
        for tci, xsrc in ((0, xTo), (1, xTh)):
            psm = psA1.tile([P, 512], F32, tag="st_mean")
            pss = psA1.tile([P, 512], F32, tag="st_sq")
            for ds in range(DS):
                nc.tensor.matmul(psm[:], ones128[:], xsrc[:, ds],
                                 start=(ds == 0), stop=(ds == DS - 1))
            for ds in range(DS):
                xq = p1c.tile([P, 512], F32R, tag="xq")
                nc.scalar.activation(xq[:], xsrc[:, ds], AF.Square)
                nc.tensor.matmul(pss[:], ones128[:], xq[:],
                                 start=(ds == 0), stop=(ds == DS - 1))
            s_, b_, m_ = _ln_coeffs(nc, p1s, psm, pss, 1.0 / DIM, eps_n[:])
            # mean row (x16 for fp8 range; matched by 1/16 in wqx/wkx/wvx)
            nc.scalar.activation(mrow[0:1, tci * 512:(tci + 1) * 512],
                                 m_[0:1, :], AF.Copy, scale=MROW_SCALE)
            # rstd as per-token columns for the v projection
            for c4 in range(4):
                ts8 = tci * 4 + c4
                pst2 = psT1.tile([P, P], F32, tag="pst2")
                nc.tensor.transpose(pst2[:],
                                    s_[:, c4 * 128:(c4 + 1) * 128],
                                    identF[:])
                nc.vector.tensor_copy(rstdT[:, ts8:ts8 + 1], pst2[:, 0:1])
    free_xTh()

    # ======================================================================
    # Phase 2: q/k/v projections (fp8 DoubleRow) + q/k LN (in-place)
    # qext[h]: rows 0..63 = q_ln (head h), row 64 = -8*slope, row 65 =
    # 8*slope*qidx. kext[g]: rows 0..63 = k_ln, row 64 = kidx, row 65 = 1.
    # vext: [tok_p, tok_sub, kv*VG] fp8 with a ones column per kv head.
    # ======================================================================
    qext, free_qext = tc.tile([P, 2, N_HEADS, NQ], F32R, name="qext")
    kext, free_kext = tc.tile([P, N_KV, T_HALO], F32R, name="kext")
    vext, free_vext = tc.tile([P, DS, N_KV * VG], FP8, name="vext")
    vview = vext[:].rearrange("p s (g e) -> p s g e", e=VG)
    # ones + pad columns (the v values fill cols 0..63 later)
    nc.vector.memset(vview[:, :, :, HD:HD + 2], 0.0)
    nc.vector.memset(vview[:, :, :, HD:HD + 1], 1.0)

    with tc.tile_pool(name="p2w", bufs=3) as p2w, \
         tc.tile_pool(name="p2c", bufs=3) as p2c, \
         tc.tile_pool(name="p2s", bufs=2) as p2s, \
         tc.tile_pool(name="psA2", bufs=1, space="PSUM") as psA2, \
         tc.tile_pool(name="psA2p", bufs=2, space="PSUM") as psA2p:

        wqxc, free_wqxc = tc.tile([2, DS, 128], FP8, name="wqxc")
        nc.sync.dma_start(wqxc[:], wqx[:])
        wkxc, free_wkxc = tc.tile([2, 4, 128], FP8, name="wkxc")
        nc.sync.dma_start(wkxc[:], wkx[:])
        # ---- q projection; LN stats batched after (no PE-queue stalls) ----
        psm = psA2.tile([P, 512], F32, tag="st_mean")
        pss = psA2.tile([P, 512], F32, tag="st_sq")
        qsqall, free_qsqall = tc.tile([HD, N_HEADS, 512], BF16, name="qsqall")
        for fs in range(DS):
            if fs % 4 == 0:
                wqc4 = p2w.tile([P, 4, DS, 128], FP8, tag="wqc")
                nc.sync.dma_start(wqc4[:], wq[fs // 4])
            wqc = wqc4[:, fs % 4]
            ps = psA2p.tile([P, 512], F32, tag="proj")
            for dp in range(DS // 2):
                nc.tensor.matmul(ps[:], wqc[:, 2 * dp:2 * dp + 2, :],
                                 x8[:, 2 * dp:2 * dp + 2, 0:T_OWN],
                                 perf_mode=DR,
                                 start=(dp == 0), stop=False)
            nc.tensor.matmul(ps[:], wqxc[:, fs], mrow[:, 0:T_OWN],
                             start=False, stop=True)
            for half in range(2):
                h = fs * 2 + half
                for t2 in range(2):
                    nc.vector.tensor_copy(
                        qext[0:HD, t2, h, :],
                        ps[half * HD:(half + 1) * HD,
                           t2 * NQ:(t2 + 1) * NQ])
                nc.scalar.activation(qsqall[:, h, :],
                                     qext[0:HD, :, h, :], AF.Square)
        for h in range(N_HEADS):
            nc.tensor.matmul(psm[:], ones128[0:HD, :], qext[0:HD, :, h, :],
                             start=(h == 0), stop=(h == N_HEADS - 1))
        for h in range(N_HEADS):
            nc.tensor.matmul(pss[:], ones_bf[0:HD, :], qsqall[:, h, :],
                             start=(h == 0), stop=(h == N_HEADS - 1))
        s_, b_, m_ = _ln_coeffs(nc, p2s, psm, pss, 1.0 / DIM, eps_qk[:])
        for h in range(N_HEADS):
            t_ = p2c.tile([HD, 512], F32R, tag="qn")
            nc.gpsimd.tensor_tensor(t_[:], qext[0:HD, :, h, :], s_[0:HD, :],
                                    OP.mult)
            nc.vector.tensor_tensor(qext[0:HD, :, h, :], t_[:], b_[0:HD, :],
                                    OP.subtract)
            if apply_qkw:
                nc.vector.tensor_scalar_mul(qext[0:HD, :, h, :],
                                            qext[0:HD, :, h, :],
                                            qkw_sb[0:HD, h:h + 1])
        nc.sync.dma_start(qext[HD:HD + 2, :, :, :], qal[:])

        # ---- k projection + interleaved k-LN stats (per token chunk) ----
        kstat = []
        for tci in range(2):
            kpsm = psA2.tile([P, 512], F32, tag=f"kst_mean{tci}")
            kpss = psA2.tile([P, 512], F32, tag=f"kst_sq{tci}")
            kstat.append((kpsm, kpss))
        wkc4 = p2w.tile([P, 4, DS, 128], FP8, tag="wkc")
        nc.scalar.dma_start(wkc4[:], wk[0])
        for fs in range(4):
            wkc = wkc4[:, fs]
            for tci in range(2):
                tsl = slice(tci * 512, (tci + 1) * 512)
                ps = psA2p.tile([P, 512], F32, tag="proj")
                for dp in range(DS // 2):
                    nc.tensor.matmul(ps[:], wkc[:, 2 * dp:2 * dp + 2, :],
                                     x8[:, 2 * dp:2 * dp + 2, tsl],
                                     perf_mode=DR,
                                     start=(dp == 0), stop=False)
                nc.tensor.matmul(ps[:], wkxc[:, fs], mrow[:, tsl],
                                 start=False, stop=True)
                for half in range(2):
                    g = fs * 2 + half
                    nc.vector.tensor_copy(kext[0:HD, g, tsl],
                                          ps[half * HD:(half + 1) * HD, :])
                    nc.scalar.activation(qsqall[:, tci * N_KV + g, :],
                                         kext[0:HD, g, tsl], AF.Square)
        for tci in range(2):
            tsl = slice(tci * 512, (tci + 1) * 512)
            psm, pss = kstat[tci]
            for g in range(N_KV):
                nc.tensor.matmul(psm[:], ones128[0:HD, :],
                                 kext[0:HD, g, tsl],
                                 start=(g == 0), stop=(g == N_KV - 1))
            for g in range(N_KV):
                nc.tensor.matmul(pss[:], ones_bf[0:HD, :],
                                 qsqall[:, tci * N_KV + g, :],
                                 start=(g == 0), stop=(g == N_KV - 1))
        kcoef = []
        for tci in range(2):
            psm, pss = kstat[tci]
            kcoef.append(_ln_coeffs(nc, p2s, psm, pss, 1.0 / (N_KV * HD),
                                    eps_qk[:]))
        for g in range(N_KV):
            for tci in range(2):
                tsl = slice(tci * 512, (tci + 1) * 512)
                s_, b_, m_ = kcoef[tci]
                t_ = p2c.tile([HD, 512], F32R, tag="kn")
                nc.gpsimd.tensor_tensor(t_[:], kext[0:HD, g, tsl],
                                        s_[0:HD, :], OP.mult)
                nc.vector.tensor_tensor(kext[0:HD, g, tsl], t_[:],
                                        b_[0:HD, :], OP.subtract)
        free_qsqall()
        for g in range(N_KV):
            nc.sync.dma_start(kext[HD:HD + 2, g, :], kal[:])

        # ---- v projection (tokens stationary, fp8 DoubleRow) ----
        wvc, free_wvc = tc.tile([P, DS, 512], FP8, name="wvc")
        nc.gpsimd.dma_start(wvc[:], wv[:])
        wvxc, free_wvxc = tc.tile([2, 512], FP8, name="wvxc")
        nc.gpsimd.dma_start(wvxc[:], wvx[:])
        for ts8 in range(DS):
            tch = slice(ts8 * 128, (ts8 + 1) * 128)
            ps = psA2p.tile([P, 512], F32, tag="proj")
            for dp in range(DS // 2):
                nc.tensor.matmul(
                    ps[:], x8[:, 2 * dp:2 * dp + 2, tch],
                    wvc[:, 2 * dp:2 * dp + 2, :], perf_mode=DR,
                    start=(dp == 0), stop=False)
            nc.tensor.matmul(ps[:], mrow[:, tch], wvxc[:],
                             start=False, stop=True)
            nc.vector.tensor_scalar_mul(
                vview[:, ts8, :, 0:HD],
                ps[:].rearrange("p (g e) -> p g e", e=HD),
                rstdT[:, ts8:ts8 + 1])
        free_wvxc()
        free_wvc()
        free_wkxc()
        free_wqxc()

    # ======================================================================
    # Phase 3: attention, head-PAIR units (8 kv-groups x 2 q-blocks of 256).
    # Both heads of a kv-group score against each K-chunk with one weight
    # load (rhs = both heads' q, N=512). Scores land in two 3-bank PSUM
    # halves so exp of half 1 overlaps scoring of half 2. AV is token-major
    # (expS stationary): av2[tok, e] carries the softmax denominator as
    # column 64 -> per-partition reciprocal + tensor_scalar normalize.
    # aoT_t [tok, feat] is PE-transposed back per pair, one pair delayed.
    # ======================================================================
    aoT_t, free_aoT_t = tc.tile([P, 4, DIM], BF16, name="aoT_t")

    def pair_scores(g, t2, half, scH):
        for kc3 in range(3):
            ks = t2 * 2 + half * 3 + kc3
            nc.tensor.matmul(
                scH[:, kc3, :],
                kext[0:HD + 2, g, ks * 128:(ks + 1) * 128],
                qext[0:HD + 2, t2, 2 * g:2 * g + 2, :],
                start=True, stop=True)

    with tc.tile_pool(name="p3", bufs=3) as p3, \
         tc.tile_pool(name="p3s", bufs=3) as p3s, \
         tc.tile_pool(name="psB1", bufs=2, space="PSUM") as psB1, \
         tc.tile_pool(name="psB2", bufs=1, space="PSUM") as psB2, \
         tc.tile_pool(name="psT", bufs=1, space="PSUM") as psT:
        pairs = [(g, t2) for g in range(N_KV) for t2 in range(2)]
        scn = psB1.tile([P, 3, 2 * NQ], F32, tag="sc")
        pair_scores(pairs[0][0], pairs[0][1], 0, scn)
        for pi, (g, t2) in enumerate(pairs):
            expS = p3.tile([P, 6, 2 * NQ], FP8, tag="expS")
            for half in range(2):
                scH = scn
                if half == 0:
                    nc.vector.tensor_tensor(scH[:, 0:2, :], scH[:, 0:2, :],
                                            mlo2[:], OP.add)
                else:
                    nc.vector.tensor_tensor(scH[:, 1:3, :], scH[:, 1:3, :],
                                            mhi2[:], OP.add)
                # next half (or next pair's first half) scores
                scn = psB1.tile([P, 3, 2 * NQ], F32, tag="sc")
                if half == 0:
                    pair_scores(g, t2, 1, scn)
                elif pi + 1 < len(pairs):
                    pair_scores(pairs[pi + 1][0], pairs[pi + 1][1], 0, scn)
                nc.scalar.activation(
                    expS[:, half * 3:(half + 1) * 3, :].rearrange(
                        "p a b -> p (a b)"),
                    scH[:].rearrange("p a b -> p (a b)"),
                    AF.Exp, scale=0.125, bias=exp_b[:])
            av2 = psB2.tile([P, 4, 72], F32, tag="av2")
            for hh in range(2):
                for qh in range(2):
                    j = hh * 2 + qh
                    for c in range(3):
                        ks = t2 * 2 + 2 * c
                        nc.tensor.matmul(
                            av2[:, j, 0:HD + 1],
                            expS[:, 2 * c:2 * c + 2,
                                 hh * NQ + qh * 128:hh * NQ + qh * 128 + 128],
                            vview[:, ks:ks + 2, g, 0:HD + 1], perf_mode=DR,
                            start=(c == 0), stop=(c == 2))
            rcp = p3s.tile([P, 4], F32, tag="rcp")
            for hh in range(2):
                for qh in range(2):
                    j = hh * 2 + qh
                    nc.vector.reciprocal_approx_fast(rcp[:, j:j + 1],
                                                     av2[:, j, HD:HD + 1])
                    nc.vector.tensor_scalar_mul(
                        aoT_t[:, t2 * 2 + qh,
                              (2 * g + hh) * HD:(2 * g + hh + 1) * HD],
                        av2[:, j, 0:HD], rcp[:, j:j + 1])
            # transpose the PREVIOUS pair (keeps the PE queue stall-free)
            for pj in (pi - 1, pi if pi == len(pairs) - 1 else -99):
                if pj < 0:
                    continue
                pg, pt2 = pairs[pj]
                for qh in range(2):
                    tb = pt2 * 2 + qh
                    pst = psT.tile([P, P], BF16, tag="pst")
                    nc.tensor.transpose(
                        pst[:], aoT_t[:, tb, pg * 128:(pg + 1) * 128],
                        ident8[:])
                    nc.vector.tensor_copy(
                        aoT[:, pg, tb * 128:(tb + 1) * 128], pst[:])
    free_aoT_t()
    free_vext()
    free_kext()
    free_qext()
    free_x8()

    # ======================================================================
    # Phase 4: wo projection (fp8 DR) + residual -> x2T ; ffn LN -> h2T
    # ======================================================================
    h2T = tile_single([P, DS, T_OWN], FP8, name="h2T")
    with tc.tile_pool(name="p4", bufs=3) as p4, \
         tc.tile_pool(name="p4s", bufs=1) as p4s, \
         tc.tile_pool(name="psC", bufs=2, space="PSUM") as psC:
        for ds2 in range(DS):
            ps = psC.tile([P, 512], F32, tag="proj")
            for kp in range(KS // 2):
                nc.tensor.matmul(
                    ps[:], woc[:, 2 * kp:2 * kp + 2,
                               ds2 * 128:(ds2 + 1) * 128],
                    aoT[:, 2 * kp:2 * kp + 2, :], perf_mode=DR,
                    start=(kp == 0), stop=(kp == KS // 2 - 1))
            nc.vector.scalar_tensor_tensor(
                x2T[:, ds2], ps[:], asc_sb[:, ds2:ds2 + 1], xTo[:, ds2],
                OP.mult, OP.add)

        # ffn LN (ffn_norm_w folded into w1/w3)
        psm = psC.tile([P, 512], F32, tag="st_mean")
        pss = psC.tile([P, 512], F32, tag="st_sq")
        for ds in range(DS):
            nc.tensor.matmul(psm[:], ones128[:], x2T[:, ds],
                             start=(ds == 0), stop=(ds == DS - 1))
        for ds in range(DS):
            xq = p4.tile([P, 512], F32R, tag="xq")
            nc.scalar.activation(xq[:], x2T[:, ds], AF.Square)
            nc.tensor.matmul(pss[:], ones128[:], xq[:],
                             start=(ds == 0), stop=(ds == DS - 1))
        s_, b_, m_ = _ln_coeffs(nc, p4s, psm, pss, 1.0 / DIM, eps_n[:])
        for ds in range(DS):
            t_ = p4.tile([P, 512], F32R, tag="t")
            nc.vector.tensor_tensor(t_[:], x2T[:, ds], m_[:], OP.subtract)
            nc.gpsimd.tensor_tensor(h2T[:, ds], t_[:], s_[:], OP.mult)

    # ======================================================================
    # Phase 5: SwiGLU FFN (fp8 DR; silu via tanh) + residual -> out
    # ======================================================================
    gT, free_gT = tc.tile([P, HS, T_OWN], FP8, name="gT")
    with tc.tile_pool(name="p5", bufs=3) as p5, \
         tc.tile_pool(name="p5w", bufs=4) as p5w, \
         tc.tile_pool(name="p5w2", bufs=2) as p5w2, \
         tc.tile_pool(name="psD", bufs=2, space="PSUM") as psD, \
         tc.tile_pool(name="psDy", bufs=2, space="PSUM") as psDy:
        NB = 4  # hs2-chunks per weight DMA (1 MB each)
        for hs2 in range(HS // 2):
            if hs2 % NB == 0:
                w1c4 = p5w.tile([P, NB, 2, DS, 128], FP8, tag="w1c")
                nc.sync.dma_start(w1c4[:], w1[hs2 // NB])
                w3c4 = p5w.tile([P, NB, 2, DS, 128], FP8, tag="w3c")
                nc.sync.dma_start(w3c4[:], w3[hs2 // NB])
            w1c = w1c4[:, hs2 % NB]
            w3c = w3c4[:, hs2 % NB]
            for half in range(2):
                hs = hs2 * 2 + half
                psu = psD.tile([P, 512], F32, tag="u")
                psw = psD.tile([P, 512], F32, tag="w")
                for dp in range(DS // 2):
                    nc.tensor.matmul(psu[:],
                                     w1c[:, half, 2 * dp:2 * dp + 2, :],
                                     h2T[:, 2 * dp:2 * dp + 2, :],
                                     perf_mode=DR,
                                     start=(dp == 0), stop=(dp == DS // 2 - 1))
                for dp in range(DS // 2):
                    nc.tensor.matmul(psw[:],
                                     w3c[:, half, 2 * dp:2 * dp + 2, :],
                                     h2T[:, 2 * dp:2 * dp + 2, :],
                                     perf_mode=DR,
                                     start=(dp == 0), stop=(dp == DS // 2 - 1))
                th = p5.tile([P, 512], F32, tag="th")
                nc.scalar.activation(th[:], psu[:], AF.Tanh,
                                     scale=1.0 / (2.0 * SW1))
                a_ = p5.tile([P, 512], F32, tag="a")
                nc.vector.scalar_tensor_tensor(a_[:], th[:], 1.0, psu[:],
                                               OP.add, OP.mult)
                nc.vector.scalar_tensor_tensor(gT[:, hs], a_[:], G_FUSE,
                                               psw[:], OP.mult, OP.mult)

        for ds2 in range(DS):
            if ds2 % 2 == 0:
                w2c2 = p5w2.tile([P, 2, HS, 128], FP8, tag="w2c")
                nc.sync.dma_start(w2c2[:], w2[ds2 // 2])
            w2c = w2c2[:, ds2 % 2]
            psy = psDy.tile([P, 512], F32, tag="y")
            for hp in range(HS // 2):
                nc.tensor.matmul(psy[:], w2c[:, 2 * hp:2 * hp + 2, :],
                                 gT[:, 2 * hp:2 * hp + 2, :], perf_mode=DR,
                                 start=(hp == 0), stop=(hp == HS // 2 - 1))
            yv = p5.tile([P, 512], F32, tag="yv")
            nc.vector.scalar_tensor_tensor(yv[:], psy[:],
                                           fsc_sb[:, ds2:ds2 + 1],
                                           x2T[:, ds2], OP.mult, OP.add)
            nc.sync.dma_start(out[:, ds2, :], yv[:])
    free_gT()
    for f in reversed(frees):
        f()


# ---------------------------------------------------------------------------
# host side
# ---------------------------------------------------------------------------

FP8_NP = ml_dtypes.float8_e4m3


def _fp8(a):
    return np.clip(a, -240.0, 240.0).astype(FP8_NP)


def _tile_kxf(wT, f_chunk):
    """[K, F] (K=contraction, multiple of 128) -> [F//f_chunk, 128, K//128,
    f_chunk] chunks whose DMA into SBUF [p, ksub, f_chunk] is contiguous."""
    K, F = wT.shape
    return np.ascontiguousarray(
        wT.reshape(K // P, P, F // f_chunk, f_chunk).transpose(2, 1, 0, 3))


def _prep_inputs(x, wq, wk, wv, wo, q_norm_w, k_norm_w, attn_norm_w,
                 ffn_norm_w, w1, w2, w3, attn_scale, ffn_scale):
    x = np.asarray(x, np.float32)
    wq = np.asarray(wq, np.float32)
    wk = np.asarray(wk, np.float32)
    wv = np.asarray(wv, np.float32)
    wo = np.asarray(wo, np.float32)
    w1 = np.asarray(w1, np.float32)
    w2 = np.asarray(w2, np.float32)
    w3 = np.asarray(w3, np.float32)
    q_norm_w = np.asarray(q_norm_w, np.float32)
    k_norm_w = np.asarray(k_norm_w, np.float32)
    attn_norm_w = np.asarray(attn_norm_w, np.float32)
    ffn_norm_w = np.asarray(ffn_norm_w, np.float32)
    attn_scale = np.asarray(attn_scale, np.float32)
    ffn_scale = np.asarray(ffn_scale, np.float32)

    # fold attn_norm into wq/wk/wv, ffn_norm into w1/w3 (column scales),
    # plus power-of-two fp8 range scaling
    wq_e = wq * attn_norm_w[None, :] * SQK
    wk_e = wk * attn_norm_w[None, :] * SQK
    wv_e = wv * attn_norm_w[None, :] * SV
    # mean-correction columns: q' = W(x - m) = Wx - m*rowsum(W)
    wqx_t = np.zeros((2, DS, 128), np.float32)
    wqx_t[0] = (-wq_e.sum(1) / MROW_SCALE).reshape(DS, 128)
    wkx_t = np.zeros((2, 4, 128), np.float32)
    wkx_t[0] = (-wk_e.sum(1) / MROW_SCALE).reshape(4, 128)
    wvx_t = np.zeros((2, 512), np.float32)
    wvx_t[0] = -wv_e.sum(1) / MROW_SCALE
    wqx_t, wkx_t, wvx_t = _fp8(wqx_t), _fp8(wkx_t), _fp8(wvx_t)
    wo_e = wo * SWO
    w1_e = w1 * ffn_norm_w[None, :] * SW1
    w3_e = w3 * ffn_norm_w[None, :] * SW3
    w2_e = w2 * SW2

    wq_t = _tile_kxf(wq_e.T, 128)                       # [8,128,8,128]
    wq_t = _fp8(np.ascontiguousarray(
        wq_t.reshape(2, 4, P, DS, 128).transpose(0, 2, 1, 3, 4)))
    wk_t = _tile_kxf(wk_e.T, 128)                       # [4,128,8,128]
    wk_t = _fp8(np.ascontiguousarray(
        wk_t.reshape(1, 4, P, DS, 128).transpose(0, 2, 1, 3, 4)))
    wv_t = _fp8(np.ascontiguousarray(
        wv_e.T.reshape(DS, P, 512).transpose(1, 0, 2)))
    wo_t = _fp8(np.ascontiguousarray(
        wo_e.T.reshape(KS, P, DIM).transpose(1, 0, 2)))
    # w1/w3: [HS//2, 128, 2, 8, 128] so one DMA per hs2 is contiguous
    w13 = []
    for w_e in (w1_e, w3_e):
        t = _tile_kxf(w_e.T, 128)                       # [32,128,8,128]
        t = t.reshape(HS // 8, 4, 2, P, DS, 128).transpose(0, 3, 1, 2, 4, 5)
        w13.append(_fp8(np.ascontiguousarray(t)))
    w1_t, w3_t = w13
    w2_t = _tile_kxf(w2_e.T, 128)                       # [8,128,32,128]
    w2_t = _fp8(np.ascontiguousarray(
        w2_t.reshape(DS // 2, 2, P, HS, 128).transpose(0, 2, 1, 3, 4)))

    # qnw*knw folded, head-local layout [p(<64), h]
    qkw = np.zeros((P, N_HEADS), np.float32)
    for h in range(N_HEADS):
        qf = h * HD + np.arange(HD)
        kf = (h // 2) * HD + np.arange(HD)
        qkw[0:HD, h] = q_norm_w[qf] * k_norm_w[kf]
    apply_qkw = not np.all(qkw[0:HD, :] == 1.0)

    def vec_tile(v):
        return np.ascontiguousarray(v.reshape(DS, P).T)

    asc = vec_tile(attn_scale / (SV * SWO))
    fsc = vec_tile(ffn_scale / (SG * SW2))

    per_core = []
    for c in range(8):
        b, blk = c // 4, c % 4
        q0 = blk * T_OWN
        hi = min(q0 + T_HALO, S)
        xblk = np.zeros((T_HALO, DIM), np.float32)
        xblk[0:hi - q0] = x[b, q0:hi]
        xT = np.ascontiguousarray(
            xblk.T.reshape(DS, P, T_HALO).transpose(1, 0, 2))
        # local indices (centered) for better f32r rounding of the rank-2
        # alibi rows; invalid halo keys get +1e9 -> logit ~ -1e9*slope
        kidx = np.arange(T_HALO, dtype=np.float32) - 512.0
        if hi - q0 < T_HALO:
            kidx[hi - q0:] += 1e9
        kal = np.stack([kidx, np.ones(T_HALO, np.float32)])
        qal = np.empty((2, N_HEADS, T_OWN), np.float32)
        for h in range(N_HEADS):
            qal[0, h, :] = -8.0 * SLOPES[h]
            qal[1, h, :] = 8.0 * SLOPES[h] * (np.arange(T_OWN) - 512.0)
        qal = np.ascontiguousarray(
            qal.reshape(2, N_HEADS, 2, 256).transpose(0, 2, 1, 3))
        per_core.append({
            "xT": xT, "x8": _fp8(xT), "wq": wq_t, "wk": wk_t, "wv": wv_t,
            "wo": wo_t, "wqx": wqx_t, "wkx": wkx_t, "wvx": wvx_t,
            "w1": w1_t, "w3": w3_t, "w2": w2_t, "qkw": qkw,
            "asc": asc, "fsc": fsc, "kal": kal, "qal": qal,
        })
    return per_core, apply_qkw


_NC_CACHE = None
LAST_RESULT = None  # BassKernelResults of the most recent run (for profiling)
TRACE = False


def kernel(**inputs):
    global _NC_CACHE, LAST_RESULT
    per_core, apply_qkw = _prep_inputs(**inputs)
    if _NC_CACHE is None:
        _NC_CACHE = _build_nc(apply_qkw)
    res = run_bass_kernel_spmd(_NC_CACHE, per_core, core_ids=list(range(8)),
                               trace=TRACE)
    LAST_RESULT = res
    full = np.empty((B, S, DIM), np.float32)
    for c in range(8):
        b, blk = c // 4, c % 4
        y = res.results[c]["out"]                      # [p, ds, tok]
        full[b, blk * T_OWN:(blk + 1) * T_OWN] = (
            y.transpose(2, 1, 0).reshape(T_OWN, DIM))
    return full


# revision 23
# speedup vs baseline: 1.0067x; 1.0067x over previous
"""Trainium2 Bass kernel for nn_CodecTransformerLayer (sparse window attention
+ GQA + ALiBi + SwiGLU FFN), 8-core data-parallel with forward-halo recompute.

Sharding: batch(2) x seq-block(4) = 8 shards, one per core. Each core computes
its own 512 tokens end-to-end; attention needs K/V for the next 512 tokens
(window is forward-looking: dist = j - i in [0, 512]), which the core
recomputes from a 512-token halo of x instead of communicating.

v2 changes vs the bf16 baseline (580us):
 - All big GEMMs (q/k/v, wo, w1/w3/w2, AV) run fp8e4 with DoubleRow perf
   mode (2 contraction rows per PE cell). Weights are pre-scaled by powers
   of two into fp8 range; the descales fold into existing affine ops
   (q/k LN is scale-invariant; attn_scale/ffn_scale absorb the rest), so
   the fp8 path costs zero extra instructions. The residual path stays
   exact fp32; the attention/FFN contributions are scaled by 1e-5 so fp8
   noise is invisible at the output.
 - FFN weights stream as fully-contiguous 2-4KB/partition DMAs (the
   baseline's interleaved 256B runs ran at ~1/8 DMA bandwidth and were the
   top stall).
 - Band masking is two DVE adds of precomputed 0/-1e9 tiles into the score
   PSUM before exp (the baseline's gpsimd affine_selects were ~6us/unit).
 - LayerNorm rstd = exp(-0.5*ln(var+eps)) keeps every activation in the
   natural_log_exp table set (no sqrt-set switches, no slow DVE
   reciprocal); softmax uses reciprocal_approx_fast on the [1,256]
   denominator row before PE-broadcast.
 - x is read as f32r directly (bit-identical to f32) so LN stats matmuls
   need no staging copies. SwiGLU's silu runs as tanh (same table set
   group) + two fused scalar_tensor_tensor DVE ops.
"""

import math

import numpy as np
import ml_dtypes

import concourse.bass as bass
import concourse.mybir as mybir
import concourse.tile as tile
from concourse import bacc
from concourse.bass_utils import run_bass_kernel_spmd

P = 128
DIM = 1024
N_HEADS = 16
N_KV = 8
HD = 64
HIDDEN = 4096
WINDOW = 512
NORM_EPS = 1e-5
QK_EPS = 1e-6
B = 2
S = 2048
T_OWN = 512          # tokens owned per core
T_HALO = 1024        # own + forward halo
DS = DIM // P        # 8 d-subtiles
KS = DIM // P        # 8 hd-subtiles for wo contraction
HS = HIDDEN // P     # 32 hidden subtiles
VG = HD + 2          # per-kv-head stride inside vext (64 v + ones + pad)

F32 = mybir.dt.float32
F32R = mybir.dt.float32r
BF16 = mybir.dt.bfloat16
FP8 = mybir.dt.float8e4
AF = mybir.ActivationFunctionType
OP = mybir.AluOpType
DR = mybir.MatmulPerfMode.DoubleRow

# power-of-two fp8 range scaling (descale locations in comments)
SQK = 64.0           # wq/wk premul; removed for free by the q/k layernorm
SV = 32.0            # wv premul; descale folded into asc
SWO = 32.0           # wo premul; descale folded into asc
SW1 = 64.0           # w1 premul; descale inside the tanh activation scale
SW3 = 64.0           # w3 premul; descale inside the g fuse constant
SG = 16.0            # gT fp8 premul; descale folded into fsc
SW2 = 64.0           # w2 premul; descale folded into fsc
G_FUSE = SG / (2.0 * SW1 * SW3)   # (th+1)*u*psw -> g scale
EXP_BIAS = -math.log(4.0)         # exp output /4 to keep fp8 < 240
MROW_SCALE = 16.0    # mean-row fp8 range boost; 1/16 folded into wqx/wkx/wvx
NEG_BIG = -1.0e9


def _alibi_slopes(n):
    start = 2.0 ** (-(2.0 ** (-(math.log2(n) - 3))))
    return [start * start ** i for i in range(n)]


SLOPES = _alibi_slopes(N_HEADS)


# ---------------------------------------------------------------------------
# device kernel
# ---------------------------------------------------------------------------

def _build_nc(apply_qkw):
    nc = bacc.Bacc("TRN2")

    ins = {}
    ins["xT"] = nc.dram_tensor("xT", [P, DS, T_HALO], F32R, kind="ExternalInput")
    ins["wq"] = nc.dram_tensor("wq", [2, P, 4, DS, 128], FP8, kind="ExternalInput")
    ins["wk"] = nc.dram_tensor("wk", [1, P, 4, DS, 128], FP8, kind="ExternalInput")
    ins["wv"] = nc.dram_tensor("wv", [P, DS, 512], FP8, kind="ExternalInput")
    ins["wo"] = nc.dram_tensor("wo", [P, KS, DIM], FP8, kind="ExternalInput")
    ins["w1"] = nc.dram_tensor("w1", [HS // 8, P, 4, 2, DS, 128], FP8,
                               kind="ExternalInput")
    ins["w3"] = nc.dram_tensor("w3", [HS // 8, P, 4, 2, DS, 128], FP8,
                               kind="ExternalInput")
    ins["w2"] = nc.dram_tensor("w2", [DS // 2, P, 2, HS, 128], FP8, kind="ExternalInput")
    # qnw*knw folded, head-local layout [64(pad128), head]
    ins["qkw"] = nc.dram_tensor("qkw", [P, N_HEADS], F32, kind="ExternalInput")
    ins["asc"] = nc.dram_tensor("asc", [P, DS], F32, kind="ExternalInput")
    ins["fsc"] = nc.dram_tensor("fsc", [P, DS], F32, kind="ExternalInput")
    ins["x8"] = nc.dram_tensor("x8", [P, DS, T_HALO], FP8,
                               kind="ExternalInput")
    ins["wqx"] = nc.dram_tensor("wqx", [2, DS, 128], FP8, kind="ExternalInput")
    ins["wkx"] = nc.dram_tensor("wkx", [2, 4, 128], FP8, kind="ExternalInput")
    ins["wvx"] = nc.dram_tensor("wvx", [2, 512], FP8, kind="ExternalInput")
    ins["kal"] = nc.dram_tensor("kal", [2, T_HALO], F32R, kind="ExternalInput")
    ins["qal"] = nc.dram_tensor("qal", [2, 2, N_HEADS, 256], F32R,
                                kind="ExternalInput")

    out = nc.dram_tensor("out", [P, DS, T_OWN], F32, kind="ExternalOutput")

    with tile.TileContext(nc) as tc:
        _emit(nc, tc, ins, out, apply_qkw)
    nc.finalize()
    return nc


def _ln_coeffs(nc, pool, psm, psq, inv_n, eps_ap):
    """From sum/sumsq psums (replicated across partitions), produce
    s = rstd and b = mean * rstd, both [128, 512] f32 replicated.
    rstd = exp(-0.5 * ln(var + eps)) keeps ACT in the ln/exp table set."""
    m_ = pool.tile([P, 512], F32, tag="ln_m")
    nc.vector.tensor_scalar_mul(m_[:], psm[:], inv_n)       # mean
    t_ = pool.tile([P, 512], F32, tag="ln_t")
    nc.vector.tensor_tensor(t_[:], m_[:], m_[:], OP.mult)   # mean^2
    d_ = pool.tile([P, 512], F32, tag="ln_d")
    nc.vector.scalar_tensor_tensor(d_[:], psq[:], inv_n, t_[:],
                                   OP.mult, OP.subtract)    # var
    s_ = pool.tile([P, 512], F32, tag="ln_s")
    nc.scalar.activation(s_[:], d_[:], AF.Ln, bias=eps_ap)
    nc.scalar.activation(s_[:], s_[:], AF.Exp, scale=-0.5)
    b_ = pool.tile([P, 512], F32, tag="ln_b")
    nc.vector.tensor_tensor(b_[:], m_[:], s_[:], OP.mult)   # mean * rstd
    return s_, b_, m_


def _emit(nc, tc, ins, out, apply_qkw):
    frees = []  # keep single-tile pool handles alive; release LIFO at end

    def tile_single(shape, dtype, name):
        t, f = tc.tile(shape, dtype, name=name)
        frees.append(f)
        return t

    xT, wq, wk, wv, wo = ins["xT"], ins["wq"], ins["wk"], ins["wv"], ins["wo"]
    x8d, wqx, wkx, wvx = ins["x8"], ins["wqx"], ins["wkx"], ins["wvx"]
    w1, w3, w2 = ins["w1"], ins["w3"], ins["w2"]
    qkw, asc, fsc = ins["qkw"], ins["asc"], ins["fsc"]
    kal, qal = ins["kal"], ins["qal"]

    # --- constants (kept for the whole kernel) -----------------------------
    ones_f = tile_single([P, P], F32, name="ones_f")
    nc.vector.memset(ones_f[:], 1.0)
    ones128 = tile_single([P, P], F32R, name="ones128")
    nc.vector.tensor_copy(ones128[:], ones_f[:])
    ones_bf = tile_single([P, P], BF16, name="ones_bf")
    nc.vector.tensor_copy(ones_bf[:], ones_f[:])
    qkw_sb = tile_single([P, N_HEADS], F32, name="qkw_sb")
    nc.sync.dma_start(qkw_sb[:], qkw[:])
    asc_sb = tile_single([P, DS], F32, name="asc_sb")
    nc.sync.dma_start(asc_sb[:], asc[:])
    fsc_sb = tile_single([P, DS], F32, name="fsc_sb")
    nc.sync.dma_start(fsc_sb[:], fsc[:])
    eps_n = tile_single([P, 1], F32, name="eps_n")
    nc.vector.memset(eps_n[:], NORM_EPS)
    eps_qk = tile_single([P, 1], F32, name="eps_qk")
    nc.vector.memset(eps_qk[:], QK_EPS)
    exp_b = tile_single([P, 1], F32, name="exp_b")
    nc.vector.memset(exp_b[:], EXP_BIAS)
    identF = tile_single([P, P], F32, name="identF")
    nc.gpsimd.memset(identF[:], 0.0)
    nc.gpsimd.affine_select(identF[:], ones_f[:], pattern=[[-1, P]], base=0,
                            channel_multiplier=1, compare_op=OP.is_equal,
                            fill=0.0)
    ident8 = tile_single([P, P], BF16, name="ident8")
    nc.vector.tensor_copy(ident8[:], identF[:])
    rstdT = tile_single([P, DS], F32, name="rstdT")
    mrow = tile_single([2, T_HALO], FP8, name="mrow")
    nc.gpsimd.memset(mrow[:], 0.0)

    # band-mask tiles: 0 in-band, -1e9 out of band. mlo covers key chunks
    # kc=0,1 (enforce dist >= 0), mhi covers kc=4,5 (enforce dist <= 512);
    # dist = kc*128 + p - c for q column c in the 256-block.
    mlo2 = tile_single([P, 2, 512], F32, name="mlo2")
    nc.gpsimd.memset(mlo2[:], 0.0)
    mhi2 = tile_single([P, 2, 512], F32, name="mhi2")
    nc.gpsimd.memset(mhi2[:], 0.0)
    for kc in (0, 1):
        for hh in range(2):
            nc.gpsimd.affine_select(
                mlo2[:, kc, hh * 256:(hh + 1) * 256],
                mlo2[:, kc, hh * 256:(hh + 1) * 256],
                pattern=[[-1, 256]], base=kc * 128, channel_multiplier=1,
                compare_op=OP.is_ge, fill=NEG_BIG)
    for kc in (4, 5):
        for hh in range(2):
            nc.gpsimd.affine_select(
                mhi2[:, kc - 4, hh * 256:(hh + 1) * 256],
                mhi2[:, kc - 4, hh * 256:(hh + 1) * 256],
                pattern=[[1, 256]], base=WINDOW - kc * 128,
                channel_multiplier=-1, compare_op=OP.is_ge, fill=NEG_BIG)

    xTo = tile_single([P, DS, T_OWN], F32R, name="xTo")
    nc.sync.dma_start(xTo[:], xT[:, :, 0:T_OWN])
    aoT = tile_single([P, KS, T_OWN], FP8, name="aoT")
    x2T = tile_single([P, DS, T_OWN], F32R, name="x2T")

    woc = tile_single([P, KS, DIM], FP8, name="woc")
    nc.gpsimd.dma_start(woc[:], wo[:])

    NQ = 256

    x8, free_x8 = tc.tile([P, DS, T_HALO], FP8, name="x8")

    # ======================================================================
    # Phase 1: attn LN over halo tokens -> hT (fp8)
    # (attn_norm_w is folded into wq/wk/wv on the host)
    # ======================================================================
    xTh, free_xTh = tc.tile([P, DS, T_OWN], F32R, name="xTh")
    nc.scalar.dma_start(xTh[:], xT[:, :, T_OWN:T_HALO])
    nc.scalar.dma_start(x8[:], x8d[:])
    with tc.tile_pool(name="p1c", bufs=3) as p1c, \
         tc.tile_pool(name="p1s", bufs=2) as p1s, \
         tc.tile_pool(name="psA1", bufs=2, space="PSUM") as psA1, \
         tc.tile_pool(name="psT1", bufs=2, space="PSUM") as psT1:
        for tci, xsrc in ((0, xTo), (1, xTh)):
            psm = psA1.tile([P, 512], F32, tag="st_mean")
            pss = psA1.tile([P, 512], F32, tag="st_sq")
            for ds in range(DS):
                nc.tensor.matmul(psm[:], ones128[:], xsrc[:, ds],
                                 start=(ds == 0), stop=(ds == DS - 1))
            for ds in range(DS):
                xq = p1c.tile([P, 512], F32R, tag="xq")
                nc.scalar.activation(xq[:], xsrc[:, ds], AF.Square)
                nc.tensor.matmul(pss[:], ones128[:], xq[:],
                                 start=(ds == 0), stop=(ds == DS - 1))
            s_, b_, m_ = _ln_coeffs(nc, p1s, psm, pss, 1.0 / DIM, eps_n[:])
            # mean row (x16 for fp8 range; matched by 1/16 in wqx/wkx/wvx)
            nc.scalar.activation(mrow[0:1, tci * 512:(tci + 1) * 512],
                                 m_[0:1, :], AF.Copy, scale=MROW_SCALE)
            # rstd as per-token columns for the v projection
            for c4 in range(4):
                ts8 = tci * 4 + c4
                pst2 = psT1.tile([P, P], F32, tag="pst2")
                nc.tensor.transpose(pst2[:],
                                    s_[:, c4 * 128:(c4 + 1) * 128],
                                    identF[:])
                nc.vector.tensor_copy(rstdT[:, ts8:ts8 + 1], pst2[:, 0:1])
    free_xTh()

    # ======================================================================
    # Phase 2: q/k/v projections (fp8 DoubleRow) + q/k LN (in-place)
    # qext[h]: rows 0..63 = q_ln (head h), row 64 = -8*slope, row 65 =
    # 8*slope*qidx. kext[g]: rows 0..63 = k_ln, row 64 = kidx, row 65 = 1.
    # vext: [tok_p, tok_sub, kv*VG] fp8 with a ones column per kv head.
    # ======================================================================
    qext, free_qext = tc.tile([P, 2, N_HEADS, NQ], F32R, name="qext")
    kext, free_kext = tc.tile([P, N_KV, T_HALO], F32R, name="kext")
    nc.sync.dma_start(qext[HD:HD + 2, :, :, :], qal[:])
    for g in range(N_KV):
        nc.sync.dma_start(kext[HD:HD + 2, g, :], kal[:])
    vext, free_vext = tc.tile([P, DS, N_KV * VG], FP8, name="vext")
    vview = vext[:].rearrange("p s (g e) -> p s g e", e=VG)
    # ones + pad columns (the v values fill cols 0..63 later)
    nc.vector.memset(vview[:, :, :, HD:HD + 2], 0.0)
    nc.vector.memset(vview[:, :, :, HD:HD + 1], 1.0)

    with tc.tile_pool(name="p2w", bufs=3) as p2w, \
         tc.tile_pool(name="p2c", bufs=3) as p2c, \
         tc.tile_pool(name="p2s", bufs=2) as p2s, \
         tc.tile_pool(name="psA2", bufs=1, space="PSUM") as psA2, \
         tc.tile_pool(name="psA2p", bufs=2, space="PSUM") as psA2p:

        wqxc, free_wqxc = tc.tile([2, DS, 128], FP8, name="wqxc")
        nc.sync.dma_start(wqxc[:], wqx[:])
        wkxc, free_wkxc = tc.tile([2, 4, 128], FP8, name="wkxc")
        nc.sync.dma_start(wkxc[:], wkx[:])
        # ---- q projection; LN stats batched after (no PE-queue stalls) ----
        psm = psA2.tile([P, 512], F32, tag="st_mean")
        pss = psA2.tile([P, 512], F32, tag="st_sq")
        qsqall, free_qsqall = tc.tile([HD, N_HEADS, 512], BF16, name="qsqall")
        for fs in range(DS):
            if fs % 4 == 0:
                wqc4 = p2w.tile([P, 4, DS, 128], FP8, tag="wqc")
                nc.sync.dma_start(wqc4[:], wq[fs // 4])
            wqc = wqc4[:, fs % 4]
            ps = psA2p.tile([P, 512], F32, tag="proj")
            for dp in range(DS // 2):
                nc.tensor.matmul(ps[:], wqc[:, 2 * dp:2 * dp + 2, :],
                                 x8[:, 2 * dp:2 * dp + 2, 0:T_OWN],
                                 perf_mode=DR,
                                 start=(dp == 0), stop=False)
            nc.tensor.matmul(ps[:], wqxc[:, fs], mrow[:, 0:T_OWN],
                             start=False, stop=True)
            for half in range(2):
                h = fs * 2 + half
                for t2 in range(2):
                    nc.vector.tensor_copy(
                        qext[0:HD, t2, h, :],
                        ps[half * HD:(half + 1) * HD,
                           t2 * NQ:(t2 + 1) * NQ])
                nc.scalar.activation(qsqall[:, h, :],
                                     qext[0:HD, :, h, :], AF.Square)
        for h in range(N_HEADS):
            nc.tensor.matmul(psm[:], ones128[0:HD, :], qext[0:HD, :, h, :],
                             start=(h == 0), stop=(h == N_HEADS - 1))
        for h in range(N_HEADS):
            nc.tensor.matmul(pss[:], ones_bf[0:HD, :], qsqall[:, h, :],
                             start=(h == 0), stop=(h == N_HEADS - 1))
        s_, b_, m_ = _ln_coeffs(nc, p2s, psm, pss, 1.0 / DIM, eps_qk[:])
        for h in range(N_HEADS):
            eng = nc.gpsimd if h % 3 == 2 else nc.vector
            t_ = p2c.tile([HD, 512], F32R, tag="qn")
            eng.tensor_tensor(t_[:], qext[0:HD, :, h, :], s_[0:HD, :],
                              OP.mult)
            eng.tensor_tensor(qext[0:HD, :, h, :], t_[:], b_[0:HD, :],
                              OP.subtract)
            if apply_qkw:
                nc.vector.tensor_scalar_mul(qext[0:HD, :, h, :],
                                            qext[0:HD, :, h, :],
                                            qkw_sb[0:HD, h:h + 1])

        # ---- k projection + interleaved k-LN stats (per token chunk) ----
        kstat = []
        for tci in range(2):
            kpsm = psA2.tile([P, 512], F32, tag=f"kst_mean{tci}")
            kpss = psA2.tile([P, 512], F32, tag=f"kst_sq{tci}")
            kstat.append((kpsm, kpss))
        wkc4 = p2w.tile([P, 4, DS, 128], FP8, tag="wkc")
        nc.scalar.dma_start(wkc4[:], wk[0])
        for fs in range(4):
            wkc = wkc4[:, fs]
            for tci in range(2):
                tsl = slice(tci * 512, (tci + 1) * 512)
                ps = psA2p.tile([P, 512], F32, tag="proj")
                for dp in range(DS // 2):
                    nc.tensor.matmul(ps[:], wkc[:, 2 * dp:2 * dp + 2, :],
                                     x8[:, 2 * dp:2 * dp + 2, tsl],
                                     perf_mode=DR,
                                     start=(dp == 0), stop=False)
                nc.tensor.matmul(ps[:], wkxc[:, fs], mrow[:, tsl],
                                 start=False, stop=True)
                for half in range(2):
                    g = fs * 2 + half
                    nc.vector.tensor_copy(kext[0:HD, g, tsl],
                                          ps[half * HD:(half + 1) * HD, :])
                    nc.scalar.activation(qsqall[:, tci * N_KV + g, :],
                                         kext[0:HD, g, tsl], AF.Square)
        for tci in range(2):
            tsl = slice(tci * 512, (tci + 1) * 512)
            psm, pss = kstat[tci]
            for g in range(N_KV):
                nc.tensor.matmul(psm[:], ones128[0:HD, :],
                                 kext[0:HD, g, tsl],
                                 start=(g == 0), stop=(g == N_KV - 1))
            for g in range(N_KV):
                nc.tensor.matmul(pss[:], ones_bf[0:HD, :],
                                 qsqall[:, tci * N_KV + g, :],
                                 start=(g == 0), stop=(g == N_KV - 1))
        kcoef = []
        for tci in range(2):
            psm, pss = kstat[tci]
            kcoef.append(_ln_coeffs(nc, p2s, psm, pss, 1.0 / (N_KV * HD),
                                    eps_qk[:]))
        for g in range(N_KV):
            for tci in range(2):
                tsl = slice(tci * 512, (tci + 1) * 512)
                s_, b_, m_ = kcoef[tci]
                eng = nc.gpsimd if (2 * g + tci) % 3 == 2 else nc.vector
                t_ = p2c.tile([HD, 512], F32R, tag="kn")
                eng.tensor_tensor(t_[:], kext[0:HD, g, tsl],
                                  s_[0:HD, :], OP.mult)
                eng.tensor_tensor(kext[0:HD, g, tsl], t_[:],
                                  b_[0:HD, :], OP.subtract)
        free_qsqall()

        # ---- v projection (tokens stationary, fp8 DoubleRow) ----
        wvc, free_wvc = tc.tile([P, DS, 512], FP8, name="wvc")
        nc.gpsimd.dma_start(wvc[:], wv[:])
        wvxc, free_wvxc = tc.tile([2, 512], FP8, name="wvxc")
        nc.gpsimd.dma_start(wvxc[:], wvx[:])
        for ts8 in range(DS):
            tch = slice(ts8 * 128, (ts8 + 1) * 128)
            ps = psA2p.tile([P, 512], F32, tag="proj")
            for dp in range(DS // 2):
                nc.tensor.matmul(
                    ps[:], x8[:, 2 * dp:2 * dp + 2, tch],
                    wvc[:, 2 * dp:2 * dp + 2, :], perf_mode=DR,
                    start=(dp == 0), stop=False)
            nc.tensor.matmul(ps[:], mrow[:, tch], wvxc[:],
                             start=False, stop=True)
            nc.vector.tensor_scalar_mul(
                vview[:, ts8, :, 0:HD],
                ps[:].rearrange("p (g e) -> p g e", e=HD),
                rstdT[:, ts8:ts8 + 1])
        free_wvxc()
        free_wvc()
        free_wkxc()
        free_wqxc()

    # ======================================================================
    # Phase 3: attention, head-PAIR units (8 kv-groups x 2 q-blocks of 256).
    # Both heads of a kv-group score against each K-chunk with one weight
    # load (rhs = both heads' q, N=512). Scores land in two 3-bank PSUM
    # halves so exp of half 1 overlaps scoring of half 2. AV is token-major
    # (expS stationary): av2[tok, e] carries the softmax denominator as
    # column 64 -> per-partition reciprocal + tensor_scalar normalize.
    # aoT_t [tok, feat] is PE-transposed back per pair, one pair delayed.
    # ======================================================================
    aoT_t, free_aoT_t = tc.tile([P, 4, DIM], BF16, name="aoT_t")

    def pair_scores(g, t2, half, scH):
        for kc3 in range(3):
            ks = t2 * 2 + half * 3 + kc3
            nc.tensor.matmul(
                scH[:, kc3, :],
                kext[0:HD + 2, g, ks * 128:(ks + 1) * 128],
                qext[0:HD + 2, t2, 2 * g:2 * g + 2, :],
                start=True, stop=True)

    with tc.tile_pool(name="p3", bufs=3) as p3, \
         tc.tile_pool(name="p3s", bufs=3) as p3s, \
         tc.tile_pool(name="psB1", bufs=2, space="PSUM") as psB1, \
         tc.tile_pool(name="psB2", bufs=1, space="PSUM") as psB2, \
         tc.tile_pool(name="psT", bufs=1, space="PSUM") as psT:
        pairs = [(g, t2) for g in range(N_KV) for t2 in range(2)]
        scn = psB1.tile([P, 3, 2 * NQ], F32, tag="sc")
        pair_scores(pairs[0][0], pairs[0][1], 0, scn)
        for pi, (g, t2) in enumerate(pairs):
            expS = p3.tile([P, 6, 2 * NQ], FP8, tag="expS")
            for half in range(2):
                scH = scn
                if half == 0:
                    nc.vector.tensor_tensor(scH[:, 0:2, :], scH[:, 0:2, :],
                                            mlo2[:], OP.add)
                else:
                    nc.vector.tensor_tensor(scH[:, 1:3, :], scH[:, 1:3, :],
                                            mhi2[:], OP.add)
                # next half (or next pair's first half) scores
                scn = psB1.tile([P, 3, 2 * NQ], F32, tag="sc")
                if half == 0:
                    pair_scores(g, t2, 1, scn)
                elif pi + 1 < len(pairs):
                    pair_scores(pairs[pi + 1][0], pairs[pi + 1][1], 0, scn)
                nc.scalar.activation(
                    expS[:, half * 3:(half + 1) * 3, :].rearrange(
                        "p a b -> p (a b)"),
                    scH[:].rearrange("p a b -> p (a b)"),
                    AF.Exp, scale=0.125, bias=exp_b[:])
            av2 = psB2.tile([P, 4, 72], F32, tag="av2")
            for hh in range(2):
                for qh in range(2):
                    j = hh * 2 + qh
                    for c in range(3):
                        ks = t2 * 2 + 2 * c
                        nc.tensor.matmul(
                            av2[:, j, 0:HD + 1],
                            expS[:, 2 * c:2 * c + 2,
                                 hh * NQ + qh * 128:hh * NQ + qh * 128 + 128],
                            vview[:, ks:ks + 2, g, 0:HD + 1], perf_mode=DR,
                            start=(c == 0), stop=(c == 2))
            rcp = p3s.tile([P, 4], F32, tag="rcp")
            for hh in range(2):
                for qh in range(2):
                    j = hh * 2 + qh
                    nc.vector.reciprocal_approx_fast(rcp[:, j:j + 1],
                                                     av2[:, j, HD:HD + 1])
                    nc.vector.tensor_scalar_mul(
                        aoT_t[:, t2 * 2 + qh,
                              (2 * g + hh) * HD:(2 * g + hh + 1) * HD],
                        av2[:, j, 0:HD], rcp[:, j:j + 1])
            # transpose the PREVIOUS pair (keeps the PE queue stall-free)
            for pj in (pi - 1, pi if pi == len(pairs) - 1 else -99):
                if pj < 0:
                    continue
                pg, pt2 = pairs[pj]
                for qh in range(2):
                    tb = pt2 * 2 + qh
                    pst = psT.tile([P, P], BF16, tag="pst")
                    nc.tensor.transpose(
                        pst[:], aoT_t[:, tb, pg * 128:(pg + 1) * 128],
                        ident8[:])
                    nc.vector.tensor_copy(
                        aoT[:, pg, tb * 128:(tb + 1) * 128], pst[:])
    free_aoT_t()
    free_vext()
    free_kext()
    free_qext()
    free_x8()

    # ======================================================================
    # Phase 4: wo projection (fp8 DR) + residual -> x2T ; ffn LN -> h2T
    # ======================================================================
    h2T = tile_single([P, DS, T_OWN], FP8, name="h2T")
    with tc.tile_pool(name="p4", bufs=3) as p4, \
         tc.tile_pool(name="p4s", bufs=1) as p4s, \
         tc.tile_pool(name="psC", bufs=2, space="PSUM") as psC:
        for ds2 in range(DS):
            ps = psC.tile([P, 512], F32, tag="proj")
            for kp in range(KS // 2):
                nc.tensor.matmul(
                    ps[:], woc[:, 2 * kp:2 * kp + 2,
                               ds2 * 128:(ds2 + 1) * 128],
                    aoT[:, 2 * kp:2 * kp + 2, :], perf_mode=DR,
                    start=(kp == 0), stop=(kp == KS // 2 - 1))
            nc.vector.scalar_tensor_tensor(
                x2T[:, ds2], ps[:], asc_sb[:, ds2:ds2 + 1], xTo[:, ds2],
                OP.mult, OP.add)

        # ffn LN (ffn_norm_w folded into w1/w3)
        psm = psC.tile([P, 512], F32, tag="st_mean")
        pss = psC.tile([P, 512], F32, tag="st_sq")
        for ds in range(DS):
            nc.tensor.matmul(psm[:], ones128[:], x2T[:, ds],
                             start=(ds == 0), stop=(ds == DS - 1))
        for ds in range(DS):
            xq = p4.tile([P, 512], F32R, tag="xq")
            nc.scalar.activation(xq[:], x2T[:, ds], AF.Square)
            nc.tensor.matmul(pss[:], ones128[:], xq[:],
                             start=(ds == 0), stop=(ds == DS - 1))
        s_, b_, m_ = _ln_coeffs(nc, p4s, psm, pss, 1.0 / DIM, eps_n[:])
        for ds in range(DS):
            t_ = p4.tile([P, 512], F32R, tag="t")
            nc.vector.tensor_tensor(t_[:], x2T[:, ds], m_[:], OP.subtract)
            nc.gpsimd.tensor_tensor(h2T[:, ds], t_[:], s_[:], OP.mult)

    # ======================================================================
    # Phase 5: SwiGLU FFN (fp8 DR; silu via tanh) + residual -> out
    # ======================================================================
    gT, free_gT = tc.tile([P, HS, T_OWN], FP8, name="gT")
    with tc.tile_pool(name="p5", bufs=3) as p5, \
         tc.tile_pool(name="p5w", bufs=4) as p5w, \
         tc.tile_pool(name="p5w2", bufs=2) as p5w2, \
         tc.tile_pool(name="psD", bufs=2, space="PSUM") as psD, \
         tc.tile_pool(name="psDy", bufs=2, space="PSUM") as psDy:
        NB = 4  # hs2-chunks per weight DMA (1 MB each)
        for hs2 in range(HS // 2):
            if hs2 % NB == 0:
                w1c4 = p5w.tile([P, NB, 2, DS, 128], FP8, tag="w1c")
                nc.sync.dma_start(w1c4[:], w1[hs2 // NB])
                w3c4 = p5w.tile([P, NB, 2, DS, 128], FP8, tag="w3c")
                nc.sync.dma_start(w3c4[:], w3[hs2 // NB])
            w1c = w1c4[:, hs2 % NB]
            w3c = w3c4[:, hs2 % NB]
            for half in range(2):
                hs = hs2 * 2 + half
                psu = psD.tile([P, 512], F32, tag="u")
                psw = psD.tile([P, 512], F32, tag="w")
                for dp in range(DS // 2):
                    nc.tensor.matmul(psu[:],
                                     w1c[:, half, 2 * dp:2 * dp + 2, :],
                                     h2T[:, 2 * dp:2 * dp + 2, :],
                                     perf_mode=DR,
                                     start=(dp == 0), stop=(dp == DS // 2 - 1))
                for dp in range(DS // 2):
                    nc.tensor.matmul(psw[:],
                                     w3c[:, half, 2 * dp:2 * dp + 2, :],
                                     h2T[:, 2 * dp:2 * dp + 2, :],
                                     perf_mode=DR,
                                     start=(dp == 0), stop=(dp == DS // 2 - 1))
                th = p5.tile([P, 512], F32, tag="th")
                nc.scalar.activation(th[:], psu[:], AF.Tanh,
                                     scale=1.0 / (2.0 * SW1))
                a_ = p5.tile([P, 512], F32, tag="a")
                nc.vector.scalar_tensor_tensor(a_[:], th[:], 1.0, psu[:],
                                               OP.add, OP.mult)
                nc.vector.scalar_tensor_tensor(gT[:, hs], a_[:], G_FUSE,
                                               psw[:], OP.mult, OP.mult)

        for ds2 in range(DS):
            if ds2 % 2 == 0:
                w2c2 = p5w2.tile([P, 2, HS, 128], FP8, tag="w2c")
                nc.sync.dma_start(w2c2[:], w2[ds2 // 2])
            w2c = w2c2[:, ds2 % 2]
            psy = psDy.tile([P, 512], F32, tag="y")
            for hp in range(HS // 2):
                nc.tensor.matmul(psy[:], w2c[:, 2 * hp:2 * hp + 2, :],
                                 gT[:, 2 * hp:2 * hp + 2, :], perf_mode=DR,
                                 start=(hp == 0), stop=(hp == HS // 2 - 1))
            yv = p5.tile([P, 512], F32, tag="yv")
            nc.vector.scalar_tensor_tensor(yv[:], psy[:],
                                           fsc_sb[:, ds2:ds2 + 1],
                                           x2T[:, ds2], OP.mult, OP.add)
            nc.sync.dma_start(out[:, ds2, :], yv[:])
    free_gT()
    for f in reversed(frees):
        f()


# ---------------------------------------------------------------------------
# host side
# ---------------------------------------------------------------------------

FP8_NP = ml_dtypes.float8_e4m3


def _fp8(a):
    return np.clip(a, -240.0, 240.0).astype(FP8_NP)


def _tile_kxf(wT, f_chunk):
    """[K, F] (K=contraction, multiple of 128) -> [F//f_chunk, 128, K//128,
    f_chunk] chunks whose DMA into SBUF [p, ksub, f_chunk] is contiguous."""
    K, F = wT.shape
    return np.ascontiguousarray(
        wT.reshape(K // P, P, F // f_chunk, f_chunk).transpose(2, 1, 0, 3))


def _prep_inputs(x, wq, wk, wv, wo, q_norm_w, k_norm_w, attn_norm_w,
                 ffn_norm_w, w1, w2, w3, attn_scale, ffn_scale):
    x = np.asarray(x, np.float32)
    wq = np.asarray(wq, np.float32)
    wk = np.asarray(wk, np.float32)
    wv = np.asarray(wv, np.float32)
    wo = np.asarray(wo, np.float32)
    w1 = np.asarray(w1, np.float32)
    w2 = np.asarray(w2, np.float32)
    w3 = np.asarray(w3, np.float32)
    q_norm_w = np.asarray(q_norm_w, np.float32)
    k_norm_w = np.asarray(k_norm_w, np.float32)
    attn_norm_w = np.asarray(attn_norm_w, np.float32)
    ffn_norm_w = np.asarray(ffn_norm_w, np.float32)
    attn_scale = np.asarray(attn_scale, np.float32)
    ffn_scale = np.asarray(ffn_scale, np.float32)

    # fold attn_norm into wq/wk/wv, ffn_norm into w1/w3 (column scales),
    # plus power-of-two fp8 range scaling
    wq_e = wq * attn_norm_w[None, :] * SQK
    wk_e = wk * attn_norm_w[None, :] * SQK
    wv_e = wv * attn_norm_w[None, :] * SV
    # mean-correction columns: q' = W(x - m) = Wx - m*rowsum(W)
    wqx_t = np.zeros((2, DS, 128), np.float32)
    wqx_t[0] = (-wq_e.sum(1) / MROW_SCALE).reshape(DS, 128)
    wkx_t = np.zeros((2, 4, 128), np.float32)
    wkx_t[0] = (-wk_e.sum(1) / MROW_SCALE).reshape(4, 128)
    wvx_t = np.zeros((2, 512), np.float32)
    wvx_t[0] = -wv_e.sum(1) / MROW_SCALE
    wqx_t, wkx_t, wvx_t = _fp8(wqx_t), _fp8(wkx_t), _fp8(wvx_t)
    wo_e = wo * SWO
    w1_e = w1 * ffn_norm_w[None, :] * SW1
    w3_e = w3 * ffn_norm_w[None, :] * SW3
    w2_e = w2 * SW2

    wq_t = _tile_kxf(wq_e.T, 128)                       # [8,128,8,128]
    wq_t = _fp8(np.ascontiguousarray(
        wq_t.reshape(2, 4, P, DS, 128).transpose(0, 2, 1, 3, 4)))
    wk_t = _tile_kxf(wk_e.T, 128)                       # [4,128,8,128]
    wk_t = _fp8(np.ascontiguousarray(
        wk_t.reshape(1, 4, P, DS, 128).transpose(0, 2, 1, 3, 4)))
    wv_t = _fp8(np.ascontiguousarray(
        wv_e.T.reshape(DS, P, 512).transpose(1, 0, 2)))
    wo_t = _fp8(np.ascontiguousarray(
        wo_e.T.reshape(KS, P, DIM).transpose(1, 0, 2)))
    # w1/w3: [HS//2, 128, 2, 8, 128] so one DMA per hs2 is contiguous
    w13 = []
    for w_e in (w1_e, w3_e):
        t = _tile_kxf(w_e.T, 128)                       # [32,128,8,128]
        t = t.reshape(HS // 8, 4, 2, P, DS, 128).transpose(0, 3, 1, 2, 4, 5)
        w13.append(_fp8(np.ascontiguousarray(t)))
    w1_t, w3_t = w13
    w2_t = _tile_kxf(w2_e.T, 128)                       # [8,128,32,128]
    w2_t = _fp8(np.ascontiguousarray(
        w2_t.reshape(DS // 2, 2, P, HS, 128).transpose(0, 2, 1, 3, 4)))

    # qnw*knw folded, head-local layout [p(<64), h]
    qkw = np.zeros((P, N_HEADS), np.float32)
    for h in range(N_HEADS):
        qf = h * HD + np.arange(HD)
        kf = (h // 2) * HD + np.arange(HD)
        qkw[0:HD, h] = q_norm_w[qf] * k_norm_w[kf]
    apply_qkw = not np.all(qkw[0:HD, :] == 1.0)

    def vec_tile(v):
        return np.ascontiguousarray(v.reshape(DS, P).T)

    asc = vec_tile(attn_scale / (SV * SWO))
    fsc = vec_tile(ffn_scale / (SG * SW2))

    per_core = []
    for c in range(8):
        b, blk = c // 4, c % 4
        q0 = blk * T_OWN
        hi = min(q0 + T_HALO, S)
        xblk = np.zeros((T_HALO, DIM), np.float32)
        xblk[0:hi - q0] = x[b, q0:hi]
        xT = np.ascontiguousarray(
            xblk.T.reshape(DS, P, T_HALO).transpose(1, 0, 2))
        # local indices (centered) for better f32r rounding of the rank-2
        # alibi rows; invalid halo keys get +1e9 -> logit ~ -1e9*slope
        kidx = np.arange(T_HALO, dtype=np.float32) - 512.0
        if hi - q0 < T_HALO:
            kidx[hi - q0:] += 1e9
        kal = np.stack([kidx, np.ones(T_HALO, np.float32)])
        qal = np.empty((2, N_HEADS, T_OWN), np.float32)
        for h in range(N_HEADS):
            qal[0, h, :] = -8.0 * SLOPES[h]
            qal[1, h, :] = 8.0 * SLOPES[h] * (np.arange(T_OWN) - 512.0)
        qal = np.ascontiguousarray(
            qal.reshape(2, N_HEADS, 2, 256).transpose(0, 2, 1, 3))
        per_core.append({
            "xT": xT, "x8": _fp8(xT), "wq": wq_t, "wk": wk_t, "wv": wv_t,
            "wo": wo_t, "wqx": wqx_t, "wkx": wkx_t, "wvx": wvx_t,
            "w1": w1_t, "w3": w3_t, "w2": w2_t, "qkw": qkw,
            "asc": asc, "fsc": fsc, "kal": kal, "qal": qal,
        })
    return per_core, apply_qkw


_NC_CACHE = None
LAST_RESULT = None  # BassKernelResults of the most recent run (for profiling)
TRACE = False


def kernel(**inputs):
    global _NC_CACHE, LAST_RESULT
    per_core, apply_qkw = _prep_inputs(**inputs)
    if _NC_CACHE is None:
        _NC_CACHE = _build_nc(apply_qkw)
    res = run_bass_kernel_spmd(_NC_CACHE, per_core, core_ids=list(range(8)),
                               trace=TRACE)
    LAST_RESULT = res
    full = np.empty((B, S, DIM), np.float32)
    for c in range(8):
        b, blk = c // 4, c % 4
        y = res.results[c]["out"]                      # [p, ds, tok]
        full[b, blk * T_OWN:(blk + 1) * T_OWN] = (
            y.transpose(2, 1, 0).reshape(T_OWN, DIM))
    return full
